# revision 63
# baseline (speedup 1.0000x reference)
"""Causal self-attention Trainium2 kernel (B=4, S=2048, D=1024, H=16).

Sharding: 8 cores = 4 batches x 2 head-groups (8 heads each).
Megatron-style: column-parallel QKV, row-parallel output projection;
the 2-way partial-sum reduce + bias happens on host at gather time.

Schedule: one interleaved PE stream. Attention for q-tile j is
software-pipelined (PE issues scores(c)/mask(c) ahead, PV lags 2
chunks so it never waits on the Act-engine exp), and the projection
tiles for group j+1 plus the output-projection tiles for group j-1
are sprinkled between attention chunks to absorb the PE slack while
Act (the per-chunk bottleneck) streams exps. Causal masking is a
-1e9 matmul accumulated into the score PSUM before the exp; softmax
denominators ride along PV as an appended ones-column block.

QKV projections run as fp8(e4m3) DoubleRow matmuls with error
feedback: x and 32*W are each split hi+lo fp8 on host and the three
significant cross terms accumulate in PSUM (xh*Wh + xh*Wl + xl*Wh),
contracting 256 channels per instruction at 0.5 cycles/row.
"""
import numpy as np
import ml_dtypes
from collections import deque
from contextlib import ExitStack

import concourse.bass as bass
import concourse.tile as tile
import concourse.mybir as mybir
from concourse.bass_utils import run_bass_kernel_spmd

B, S, D, H = 4, 2048, 1024, 16
HD = 64          # head dim
HPC = 8          # heads per core
DG = HPC * HD    # 512 dims per head-group
P = 128
NQ = 512         # q-tile width
NCH = S // P     # 16 k-chunks
NJ = S // NQ     # 4 q-tiles (= j-groups)
CP = 4           # 256-channel chunk-pairs over D (fp8 DoubleRow)
DT = mybir.dt.bfloat16
F8 = mybir.dt.float8e4
NPDT = ml_dtypes.bfloat16
NPF8 = ml_dtypes.float8_e4m3

WS = 32.0               # fp8 weight pre-scale (host side)
SCL = WS * WS           # scores carry WS^2; exp scale divides it out

_CACHE = {}


def split_waits(nc, maxw=1):
    """walrus here accepts at most 1 sync-wait per instruction; split extras onto NOPs."""
    for fn in nc.m.functions:
        for bb in fn.blocks:
            insts = list(bb.instructions)
            new_list = []
            changed = False
            for inst in insts:
                si = inst.sync_info
                waits = list(si.on_wait) if si and si.on_wait else []
                if len(waits) > maxw:
                    changed = True
                    head, keep = waits[:-maxw], waits[-maxw:]
                    for i in range(0, len(head), maxw):
                        nop = mybir.InstNoOp(
                            name=f"{inst.name}_wsplit{i}",
                            sync_info=mybir.SyncInfo(on_wait=head[i:i + maxw], on_update=[]),
                            bass_nofuse=True, engine=inst.engine)
                        nc.register_instruction(nop)
                        new_list.append(nop)
                    inst.sync_info = mybir.SyncInfo(
                        on_wait=keep,
                        on_update=list(si.on_update) if si.on_update else [])
                new_list.append(inst)
            if changed:
                bb.instructions = new_list


def build():
    nc = bass.Bass(trn_type="TRN2", target_bir_lowering=False, debug=False)
    xh = nc.dram_tensor("xh", [P, CP, 2, S], F8, kind="ExternalInput").ap()
    xl = nc.dram_tensor("xl", [P, CP, 2, S], F8, kind="ExternalInput").ap()
    wqkv = {}
    for nm in ("wqh", "wql", "wkh", "wkl", "wvh", "wvl"):
        wqkv[nm] = nc.dram_tensor(nm, [P, CP, 2, DG], F8, kind="ExternalInput").ap()
    wo8h = nc.dram_tensor("wo8h", [P, 2, 2, D], F8, kind="ExternalInput").ap()
    wo8l = nc.dram_tensor("wo8l", [P, 2, 2, D], F8, kind="ExternalInput").ap()
    wo23 = nc.dram_tensor("wo23", [P, 2, D], DT, kind="ExternalInput").ap()
    mnegT = nc.dram_tensor("mnegT", [P, P], DT, kind="ExternalInput").ap()
    ident = nc.dram_tensor("ident", [P, P], DT, kind="ExternalInput").ap()
    y = nc.dram_tensor("y", [S, D], mybir.dt.float16, kind="ExternalOutput").ap()

    with tile.TileContext(nc) as tc, ExitStack() as ctx:
        const = ctx.enter_context(tc.tile_pool(name="const", bufs=1))
        xw = ctx.enter_context(tc.tile_pool(name="xw", bufs=1))
        acts = ctx.enter_context(tc.tile_pool(name="acts", bufs=1))

        # ---- resident inputs; DMA order gates the pipeline fill ----
        xh_sb = xw.tile([P, CP, 2, S], F8)
        xl_sb = xw.tile([P, CP, 2, S], F8)
        w_sb = {}

        def load_w(nm):
            w_sb[nm] = xw.tile([P, CP, 2, DG], F8, name=nm)
            nc.sync.dma_start(w_sb[nm][:], wqkv[nm][:])

        def load_xq(g):  # x S-quarter g: all group-g projection inputs
            for c in range(CP):
                nc.sync.dma_start(xh_sb[:, c, :, bass.ts(g, NQ)], xh[:, c, :, bass.ts(g, NQ)])
                nc.sync.dma_start(xl_sb[:, c, :, bass.ts(g, NQ)], xl[:, c, :, bass.ts(g, NQ)])

        load_w("wqh")
        for c in range(CP):
            nc.sync.dma_start(xh_sb[:, c, :, 0:NQ], xh[:, c, :, 0:NQ])
        load_w("wql")
        for c in range(CP):
            nc.sync.dma_start(xl_sb[:, c, :, 0:NQ], xl[:, c, :, 0:NQ])
        for nm in ("wkh", "wkl", "wvh", "wvl"):
            load_w(nm)
        mneg_sb = const.tile([P, P], DT)
        nc.sync.dma_start(mneg_sb[:], mnegT[:])
        id_sb = const.tile([P, P], DT)
        nc.sync.dma_start(id_sb[:], ident[:])
        for g in range(1, NJ):    # remaining x quarters stream under compute
            load_xq(g)
        wo8h_sb = xw.tile([P, 2, 2, D], F8)
        nc.sync.dma_start(wo8h_sb[:], wo8h[:])
        wo8l_sb = xw.tile([P, 2, 2, D], F8)
        nc.sync.dma_start(wo8l_sb[:], wo8l[:])
        wo23_sb = xw.tile([P, 2, D], DT)
        nc.sync.dma_start(wo23_sb[:], wo23[:])

        # ---- resident activations ----
        qT_sb = acts.tile([P, DG // P, S], DT)   # [2-head block, hi, s]
        kT_sb = acts.tile([P, DG // P, S], DT)
        v_sb = acts.tile([P, NCH, HPC, P], DT)   # [k part, chunk, head, V|ones]
        nc.vector.memset(v_sb[:, :, :, HD:], 1.0)
        oT_sb = acts.tile([P, DG // P, S], DT)
        oh8_sb = acts.tile([P, 2, 2, S], F8)   # DoubleRow lhsT: dg = c2*256+i*128+p
        ol8_sb = acts.tile([P, 2, 2, S], F8)

        spool = ctx.enter_context(tc.tile_pool(name="sp", bufs=2, space="PSUM"))
        pqpool = ctx.enter_context(tc.tile_pool(name="pq", bufs=2, space="PSUM"))
        pospool = ctx.enter_context(tc.tile_pool(name="po", bufs=1, space="PSUM"))
        pt = ctx.enter_context(tc.tile_pool(name="pt", bufs=4))
        rc = ctx.enter_context(tc.tile_pool(name="rc", bufs=2))
        ys = ctx.enter_context(tc.tile_pool(name="ys", bufs=6))

        def emit_qk_tile(dst_sb, kind, i, g, on_act=False):
            """One [128, 512] tile of Q^T/K^T (d-block i, q-cols of group g)."""
            ps = pqpool.tile([P, NQ], mybir.dt.float32, tag="pp")
            wh, wl = w_sb["w%sh" % kind], w_sb["w%sl" % kind]
            k = 0
            for wt, xt in ((wh, xh_sb), (wl, xh_sb), (wh, xl_sb)):
                for c in range(CP):
                    nc.tensor.matmul(
                        ps[:], wt[:, c, :, bass.ts(i, P)],
                        xt[:, c, :, bass.ts(g, NQ)],
                        start=(k == 0), stop=(k == 3 * CP - 1),
                        perf_mode=mybir.MatmulPerfMode.DoubleRow)
                    k += 1
            if on_act:
                nc.scalar.copy(dst_sb[:, i, bass.ts(g, NQ)], ps[:])
            else:
                nc.vector.tensor_copy(dst_sb[:, i, bass.ts(g, NQ)], ps[:])

        def emit_v_tile(m, on_act=False):
            """V s-block m -> v_sb[:, m, :, 0:HD] (scaled back by 1/WS)."""
            ps = pqpool.tile([P, DG], mybir.dt.float32, tag="pp")
            wh, wl = w_sb["wvh"], w_sb["wvl"]
            k = 0
            for xt, wt in ((xh_sb, wh), (xh_sb, wl), (xl_sb, wh)):
                for c in range(CP):
                    nc.tensor.matmul(
                        ps[:], xt[:, c, :, bass.ts(m, P)], wt[:, c],
                        start=(k == 0), stop=(k == 3 * CP - 1),
                        perf_mode=mybir.MatmulPerfMode.DoubleRow)
                    k += 1
            if on_act:
                nc.scalar.mul(v_sb[:, m, :, 0:HD],
                              ps[:].rearrange("p (h d) -> p h d", d=HD), 1.0 / WS)
            else:
                nc.vector.tensor_scalar_mul(
                    v_sb[:, m, :, 0:HD],
                    ps[:].rearrange("p (h d) -> p h d", d=HD), 1.0 / WS)

        def emit_outproj_tile(m, n, on_act=False, pool=None):
            ps = (pool or pqpool).tile([P, NQ], mybir.dt.float32,
                                       tag="pp" if pool is None else "ps")
            k = 0
            for c2 in range(2):
                for ot, wt in ((oh8_sb, wo8h_sb), (oh8_sb, wo8l_sb),
                               (ol8_sb, wo8h_sb)):
                    nc.tensor.matmul(
                        ps[:], ot[:, c2, :, bass.ts(m, P)],
                        wt[:, c2, :, bass.ts(n, NQ)],
                        start=(k == 0), stop=(k == 5),
                        perf_mode=mybir.MatmulPerfMode.DoubleRow)
                    k += 1
            ysb = ys.tile([P, NQ], mybir.dt.float16, tag="ysb")
            if on_act:   # tail: Act is idle once the exps are done
                nc.scalar.mul(ysb[:], ps[:], 1.0 / WS)
            else:
                nc.vector.tensor_scalar_mul(ysb[:], ps[:], 1.0 / WS)
            nc.sync.dma_start(y[bass.ts(m, P), bass.ts(n, NQ)], ysb[:])

        def proj_extras(g):
            # groups 1/2 run inside attention(0)/(1) where Act has slack;
            # evict via Act there to relieve the saturated DVE
            oa = g <= 2
            ex = []
            for i in range(DG // P):
                ex.append(lambda i=i: emit_qk_tile(qT_sb, "q", i, g, on_act=oa))
            for i in range(DG // P):
                ex.append(lambda i=i: emit_qk_tile(kT_sb, "k", i, g, on_act=oa))
            for m in range(4 * g, 4 * g + 4):
                ex.append(lambda m=m: emit_v_tile(m, on_act=oa))
            return ex

        def outproj_extras(g):
            return [lambda m=m, n=n: emit_outproj_tile(m, n)
                    for m in range(4 * g, 4 * g + 4) for n in range(D // NQ)]

        def emit_pv(pos, hi, nch, c, qo, pT):
            for s in range(2):
                nc.tensor.matmul(
                    pos[:, s, qo:NQ], v_sb[:, c, 2 * hi + s, :],
                    pT[:, s, qo:NQ],
                    start=(c == 0), stop=(c == nch - 1))

        def emit_attention_group(j, extras, last=False, v_inline=None):
            """All 4 head-pairs for q-tile j, extras paced over the chunks."""
            nch = 4 * j + 4
            total_c = nch * (HPC // 2)
            n_extras = len(extras)
            ci = 0
            done_extras = 0
            for hi in range(HPC // 2):
                pos = pospool.tile([P, 2, NQ], mybir.dt.float32, tag="pos")
                pend = []  # (c, qo, pT)
                for c in range(nch):
                    qo = max(0, P * c - NQ * j)
                    ps = spool.tile([P, 2, NQ], mybir.dt.float32, tag="ps")
                    diag = c >= 4 * j
                    for s in range(2):
                        hb = s * HD
                        nc.tensor.matmul(
                            ps[:, s, qo:NQ],
                            kT_sb[hb:hb + HD, hi, bass.ts(c, P)],
                            qT_sb[hb:hb + HD, hi, NQ * j + qo:NQ * (j + 1)],
                            start=True, stop=True)
                    pT = pt.tile([P, 2, NQ], DT, tag="pT")
                    nc.scalar.activation(
                        pT[:, :, qo:NQ], ps[:, :, qo:NQ],
                        mybir.ActivationFunctionType.Exp,
                        scale=float(HD) ** -0.5 / SCL)
                    if diag:
                        # causal triangle on the diagonal block: in-place
                        # bf16 multiply on DVE (PV lags 2 chunks, so the
                        # cross-engine hop stays off the critical path)
                        for s in range(2):
                            nc.vector.tensor_tensor(
                                pT[:, s, qo:qo + P], pT[:, s, qo:qo + P],
                                mneg_sb[:], mybir.AluOpType.mult)
                    pend.append((c, qo, pT))
                    if len(pend) > 4:
                        emit_pv(pos, hi, nch, *pend.pop(0))
                    if v_inline and hi == 0 and c < len(v_inline):
                        v_inline[c]()
                    ci += 1
                    # pace extras to deplete slightly after the last chunk so
                    # a few remain to cover the final exp->PV drain
                    while extras and done_extras * (total_c + 20) < ci * n_extras:
                        extras.popleft()()
                        done_extras += 1
                while pend:
                    emit_pv(pos, hi, nch, *pend.pop(0))
                    if extras and hi == HPC // 2 - 1:
                        extras.popleft()()
                rcp = rc.tile([P, 2, NQ], mybir.dt.float32, tag="rcp")
                if last and hi == HPC // 2 - 1:
                    # tail: normalize straight from PSUM (shortest oT chain)
                    nc.vector.reciprocal(rcp[HD:P, :, :], pos[HD:P, :, :])
                    for s in range(2):
                        nc.vector.tensor_tensor(
                            oT_sb[s * HD:(s + 1) * HD, hi, bass.ts(j, NQ)],
                            pos[0:HD, s, :], rcp[HD:P, s, :], mybir.AluOpType.mult)
                else:
                    # free the pos bank fast: evict to SBUF, normalize there.
                    # rcp lands on partitions 0:HD so the SBUF+SBUF multiply
                    # reads both inputs at the same base partition (walrus
                    # requires equal SBUF base partitions).
                    posE = rc.tile([P, 2, NQ], mybir.dt.float32, tag="posE")
                    nc.vector.tensor_copy(posE[:], pos[:])
                    nc.vector.reciprocal(rcp[0:HD, :, :], posE[HD:P, :, :])
                    for s in range(2):
                        nc.vector.tensor_tensor(
                            oT_sb[s * HD:(s + 1) * HD, hi, bass.ts(j, NQ)],
                            posE[0:HD, s, :], rcp[0:HD, s, :], mybir.AluOpType.mult)
                if last and hi >= 2:
                    continue   # tail outproj reads oT (bf16) for hi 2/3
                eng = nc.gpsimd
                eng.tensor_copy(
                    oh8_sb[:, hi // 2, hi % 2, bass.ts(j, NQ)],
                    oT_sb[:, hi, bass.ts(j, NQ)])
                eng.tensor_tensor(
                    ol8_sb[:, hi // 2, hi % 2, bass.ts(j, NQ)],
                    oT_sb[:, hi, bass.ts(j, NQ)],
                    oh8_sb[:, hi // 2, hi % 2, bass.ts(j, NQ)],
                    mybir.AluOpType.subtract)
            while extras:
                extras.popleft()()

        # ---- interleaved schedule ----
        # proj(g+1) extras must finish inside attention(g); outproj extras are
        # free to defer, so they all go to attention(3) whose Act deficit is
        # largest. outproj(3) trails as the unavoidable tail.
        for i in range(DG // P):
            emit_qk_tile(qT_sb, "q", i, 0)
        for i in range(DG // P):
            emit_qk_tile(kT_sb, "k", i, 0)
        v0 = [lambda m=m: emit_v_tile(m) for m in range(4)]
        for g in range(NJ):
            extras = deque()
            if g + 1 < NJ:
                extras.extend(proj_extras(g + 1))
            if g == NJ - 1:
                for gg in range(NJ - 1):
                    extras.extend(outproj_extras(gg))
            emit_attention_group(g, extras, last=(g == NJ - 1),
                                 v_inline=v0 if g == 0 else None)
        for mi, m in enumerate(range(4 * (NJ - 1), 4 * NJ)):
            ysb2 = ys.tile([P, 2, NQ], mybir.dt.float16, tag="ysb2", name="ysb2")
            for n in range(D // NQ):
                ps = (spool if n else pqpool).tile(
                    [P, NQ], mybir.dt.float32, tag="ps" if n else "pp",
                    name="pst")
                k = 0
                for ot, wt in ((oh8_sb, wo8h_sb), (oh8_sb, wo8l_sb),
                               (ol8_sb, wo8h_sb)):
                    nc.tensor.matmul(
                        ps[:], ot[:, 0, :, bass.ts(m, P)],
                        wt[:, 0, :, bass.ts(n, NQ)],
                        start=(k == 0), stop=False,
                        perf_mode=mybir.MatmulPerfMode.DoubleRow)
                    k += 1
                for c in range(2, 4):
                    nc.tensor.matmul(
                        ps[:], oT_sb[:, c, bass.ts(m, P)],
                        wo23_sb[:, c - 2, bass.ts(n, NQ)],
                        start=False, stop=(c == 3))
                if n:
                    nc.scalar.mul(ysb2[:, n, :], ps[:], 1.0 / WS)
                else:
                    nc.vector.tensor_scalar_mul(ysb2[:, n, :], ps[:], 1.0 / WS)
                if mi == 3:   # last tile: per-half DMA so the first half
                    nc.sync.dma_start(   # transfers under the second evict
                        y[bass.ts(m, P), n * NQ:(n + 1) * NQ], ysb2[:, n, :])
            if mi < 3:
                eng = nc.scalar if mi % 2 else nc.sync
                eng.dma_start(y[bass.ts(m, P), :], ysb2[:])

    split_waits(nc)
    return nc


def kernel(x, Wq, Wk, Wv, Wo, bo):
    x, Wq, Wk, Wv, Wo, bo = (np.asarray(a, np.float32) for a in (x, Wq, Wk, Wv, Wo, bo))
    if "nc" not in _CACHE:
        _CACHE["nc"] = build()
    nc = _CACHE["nc"]

    # causal keep-mask for the diagonal block: tri[k, q] = 1 where q >= k
    mnegT = np.triu(np.ones((P, P), np.float32)).astype(NPDT)
    ident = np.eye(P, dtype=np.float32).astype(NPDT)

    def dr_pack(a):  # [1024, N] -> [128, 4, 2, N] with channel = c*256 + i*128 + p
        return np.ascontiguousarray(a.reshape(CP, 2, P, -1).transpose(2, 0, 1, 3))

    in_maps = []
    for core in range(8):
        b, gsl = core // 2, core % 2
        sl = slice(gsl * DG, (gsl + 1) * DG)
        xt = np.ascontiguousarray(x[b].T)
        xh8 = xt.astype(NPF8)
        xl8 = (xt - xh8.astype(np.float32)).astype(NPF8)
        wos = Wo[sl, :] * WS
        wo23_np = np.ascontiguousarray(
            wos[2 * P:4 * P].reshape(2, P, D).transpose(1, 0, 2)).astype(NPDT)
        woh = wos.astype(NPF8)
        wol = (wos - woh.astype(np.float32)).astype(NPF8)
        pk = lambda a: np.ascontiguousarray(a.reshape(2, 2, P, D).transpose(2, 0, 1, 3))
        m = {"wo8h": pk(woh), "wo8l": pk(wol), "wo23": wo23_np,
             "mnegT": mnegT, "ident": ident,
             "xh": dr_pack(xh8), "xl": dr_pack(xl8)}
        for nm, W in (("q", Wq), ("k", Wk), ("v", Wv)):
            ws = W[:, sl] * WS
            wh = ws.astype(NPF8)
            wl = (ws - wh.astype(np.float32)).astype(NPF8)
            m["w%sh" % nm] = dr_pack(wh)
            m["w%sl" % nm] = dr_pack(wl)
        in_maps.append(m)
    res = run_bass_kernel_spmd(nc, in_maps, list(range(8)))
    out = np.empty((B, S, D), np.float32)
    for b in range(B):
        out[b] = (res.results[2 * b]["y"].astype(np.float32)
                  + res.results[2 * b + 1]["y"].astype(np.float32) + bo)
    return out


# revision 66
# speedup vs baseline: 1.0011x; 1.0011x over previous
"""Causal self-attention Trainium2 kernel (B=4, S=2048, D=1024, H=16).

Sharding: 8 cores = 4 batches x 2 head-groups (8 heads each).
Megatron-style: column-parallel QKV, row-parallel output projection;
the 2-way partial-sum reduce + bias happens on host at gather time.

Schedule: one interleaved PE stream. Attention for q-tile j is
software-pipelined (PE issues scores(c)/mask(c) ahead, PV lags 2
chunks so it never waits on the Act-engine exp), and the projection
tiles for group j+1 plus the output-projection tiles for group j-1
are sprinkled between attention chunks to absorb the PE slack while
Act (the per-chunk bottleneck) streams exps. Causal masking is a
-1e9 matmul accumulated into the score PSUM before the exp; softmax
denominators ride along PV as an appended ones-column block.

QKV projections run as fp8(e4m3) DoubleRow matmuls with error
feedback: x and 32*W are each split hi+lo fp8 on host and the three
significant cross terms accumulate in PSUM (xh*Wh + xh*Wl + xl*Wh),
contracting 256 channels per instruction at 0.5 cycles/row.
"""
import numpy as np
import ml_dtypes
from collections import deque
from contextlib import ExitStack

import concourse.bass as bass
import concourse.tile as tile
import concourse.mybir as mybir
from concourse.bass_utils import run_bass_kernel_spmd

B, S, D, H = 4, 2048, 1024, 16
HD = 64          # head dim
HPC = 8          # heads per core
DG = HPC * HD    # 512 dims per head-group
P = 128
NQ = 512         # q-tile width
NCH = S // P     # 16 k-chunks
NJ = S // NQ     # 4 q-tiles (= j-groups)
CP = 4           # 256-channel chunk-pairs over D (fp8 DoubleRow)
DT = mybir.dt.bfloat16
F8 = mybir.dt.float8e4
NPDT = ml_dtypes.bfloat16
NPF8 = ml_dtypes.float8_e4m3

WS = 32.0               # fp8 weight pre-scale (host side)
SCL = WS * WS           # scores carry WS^2; exp scale divides it out

_CACHE = {}


def split_waits(nc, maxw=1):
    """walrus here accepts at most 1 sync-wait per instruction; split extras onto NOPs."""
    for fn in nc.m.functions:
        for bb in fn.blocks:
            insts = list(bb.instructions)
            new_list = []
            changed = False
            for inst in insts:
                si = inst.sync_info
                waits = list(si.on_wait) if si and si.on_wait else []
                if len(waits) > maxw:
                    changed = True
                    head, keep = waits[:-maxw], waits[-maxw:]
                    for i in range(0, len(head), maxw):
                        nop = mybir.InstNoOp(
                            name=f"{inst.name}_wsplit{i}",
                            sync_info=mybir.SyncInfo(on_wait=head[i:i + maxw], on_update=[]),
                            bass_nofuse=True, engine=inst.engine)
                        nc.register_instruction(nop)
                        new_list.append(nop)
                    inst.sync_info = mybir.SyncInfo(
                        on_wait=keep,
                        on_update=list(si.on_update) if si.on_update else [])
                new_list.append(inst)
            if changed:
                bb.instructions = new_list


def build():
    nc = bass.Bass(trn_type="TRN2", target_bir_lowering=False, debug=False)
    xh = nc.dram_tensor("xh", [P, CP, 2, S], F8, kind="ExternalInput").ap()
    xl = nc.dram_tensor("xl", [P, CP, 2, S], F8, kind="ExternalInput").ap()
    wqkv = {}
    for nm in ("wqh", "wql", "wkh", "wkl", "wvh", "wvl"):
        wqkv[nm] = nc.dram_tensor(nm, [P, CP, 2, DG], F8, kind="ExternalInput").ap()
    wo8h = nc.dram_tensor("wo8h", [P, 2, 2, D], F8, kind="ExternalInput").ap()
    wo8l = nc.dram_tensor("wo8l", [P, 2, 2, D], F8, kind="ExternalInput").ap()
    wo23 = nc.dram_tensor("wo23", [P, 2, D], DT, kind="ExternalInput").ap()
    mnegT = nc.dram_tensor("mnegT", [P, P], DT, kind="ExternalInput").ap()
    ident = nc.dram_tensor("ident", [P, P], DT, kind="ExternalInput").ap()
    y = nc.dram_tensor("y", [S, D], mybir.dt.float16, kind="ExternalOutput").ap()

    with tile.TileContext(nc) as tc, ExitStack() as ctx:
        const = ctx.enter_context(tc.tile_pool(name="const", bufs=1))
        xw = ctx.enter_context(tc.tile_pool(name="xw", bufs=1))
        acts = ctx.enter_context(tc.tile_pool(name="acts", bufs=1))

        # ---- resident inputs; DMA order gates the pipeline fill ----
        xh_sb = xw.tile([P, CP, 2, S], F8)
        xl_sb = xw.tile([P, CP, 2, S], F8)
        w_sb = {}

        def load_w(nm):
            w_sb[nm] = xw.tile([P, CP, 2, DG], F8, name=nm)
            nc.sync.dma_start(w_sb[nm][:], wqkv[nm][:])

        def load_xq(g):  # x S-quarter g: all group-g projection inputs
            for c in range(CP):
                nc.sync.dma_start(xh_sb[:, c, :, bass.ts(g, NQ)], xh[:, c, :, bass.ts(g, NQ)])
                nc.sync.dma_start(xl_sb[:, c, :, bass.ts(g, NQ)], xl[:, c, :, bass.ts(g, NQ)])

        load_w("wqh")
        for c in range(CP):
            nc.sync.dma_start(xh_sb[:, c, :, 0:NQ], xh[:, c, :, 0:NQ])
        load_w("wql")
        for c in range(CP):
            nc.sync.dma_start(xl_sb[:, c, :, 0:NQ], xl[:, c, :, 0:NQ])
        for nm in ("wkh", "wkl", "wvh", "wvl"):
            load_w(nm)
        mneg_sb = const.tile([P, P], DT)
        nc.sync.dma_start(mneg_sb[:], mnegT[:])
        id_sb = const.tile([P, P], DT)
        nc.sync.dma_start(id_sb[:], ident[:])
        for g in range(1, NJ):    # remaining x quarters stream under compute
            load_xq(g)
        wo8h_sb = xw.tile([P, 2, 2, D], F8)
        nc.sync.dma_start(wo8h_sb[:], wo8h[:])
        wo8l_sb = xw.tile([P, 2, 2, D], F8)
        nc.sync.dma_start(wo8l_sb[:], wo8l[:])
        wo23_sb = xw.tile([P, 2, D], DT)
        nc.sync.dma_start(wo23_sb[:], wo23[:])

        # ---- resident activations ----
        qT_sb = acts.tile([P, DG // P, S], DT)   # [2-head block, hi, s]
        kT_sb = acts.tile([P, DG // P, S], DT)
        v_sb = acts.tile([P, NCH, HPC, P], DT)   # [k part, chunk, head, V|ones]
        nc.vector.memset(v_sb[:, :, :, HD:], 1.0)
        oT_sb = acts.tile([P, DG // P, S], DT)
        oh8_sb = acts.tile([P, 2, 2, S], F8)   # DoubleRow lhsT: dg = c2*256+i*128+p
        ol8_sb = acts.tile([P, 2, 2, S], F8)

        spool = ctx.enter_context(tc.tile_pool(name="sp", bufs=2, space="PSUM"))
        pqpool = ctx.enter_context(tc.tile_pool(name="pq", bufs=2, space="PSUM"))
        pospool = ctx.enter_context(tc.tile_pool(name="po", bufs=1, space="PSUM"))
        pt = ctx.enter_context(tc.tile_pool(name="pt", bufs=5))
        rc = ctx.enter_context(tc.tile_pool(name="rc", bufs=1))
        ys = ctx.enter_context(tc.tile_pool(name="ys", bufs=6))

        def emit_qk_tile(dst_sb, kind, i, g, on_act=False):
            """One [128, 512] tile of Q^T/K^T (d-block i, q-cols of group g)."""
            ps = pqpool.tile([P, NQ], mybir.dt.float32, tag="pp")
            wh, wl = w_sb["w%sh" % kind], w_sb["w%sl" % kind]
            k = 0
            for wt, xt in ((wh, xh_sb), (wl, xh_sb), (wh, xl_sb)):
                for c in range(CP):
                    nc.tensor.matmul(
                        ps[:], wt[:, c, :, bass.ts(i, P)],
                        xt[:, c, :, bass.ts(g, NQ)],
                        start=(k == 0), stop=(k == 3 * CP - 1),
                        perf_mode=mybir.MatmulPerfMode.DoubleRow)
                    k += 1
            if on_act:
                nc.scalar.copy(dst_sb[:, i, bass.ts(g, NQ)], ps[:])
            else:
                nc.vector.tensor_copy(dst_sb[:, i, bass.ts(g, NQ)], ps[:])

        def emit_v_tile(m, on_act=False):
            """V s-block m -> v_sb[:, m, :, 0:HD] (scaled back by 1/WS)."""
            ps = pqpool.tile([P, DG], mybir.dt.float32, tag="pp")
            wh, wl = w_sb["wvh"], w_sb["wvl"]
            k = 0
            for xt, wt in ((xh_sb, wh), (xh_sb, wl), (xl_sb, wh)):
                for c in range(CP):
                    nc.tensor.matmul(
                        ps[:], xt[:, c, :, bass.ts(m, P)], wt[:, c],
                        start=(k == 0), stop=(k == 3 * CP - 1),
                        perf_mode=mybir.MatmulPerfMode.DoubleRow)
                    k += 1
            if on_act:
                nc.scalar.mul(v_sb[:, m, :, 0:HD],
                              ps[:].rearrange("p (h d) -> p h d", d=HD), 1.0 / WS)
            else:
                nc.vector.tensor_scalar_mul(
                    v_sb[:, m, :, 0:HD],
                    ps[:].rearrange("p (h d) -> p h d", d=HD), 1.0 / WS)

        def emit_outproj_tile(m, n, on_act=False, pool=None):
            ps = (pool or pqpool).tile([P, NQ], mybir.dt.float32,
                                       tag="pp" if pool is None else "ps")
            k = 0
            for c2 in range(2):
                for ot, wt in ((oh8_sb, wo8h_sb), (oh8_sb, wo8l_sb),
                               (ol8_sb, wo8h_sb)):
                    nc.tensor.matmul(
                        ps[:], ot[:, c2, :, bass.ts(m, P)],
                        wt[:, c2, :, bass.ts(n, NQ)],
                        start=(k == 0), stop=(k == 5),
                        perf_mode=mybir.MatmulPerfMode.DoubleRow)
                    k += 1
            ysb = ys.tile([P, NQ], mybir.dt.float16, tag="ysb")
            if on_act:   # tail: Act is idle once the exps are done
                nc.scalar.mul(ysb[:], ps[:], 1.0 / WS)
            else:
                nc.vector.tensor_scalar_mul(ysb[:], ps[:], 1.0 / WS)
            nc.sync.dma_start(y[bass.ts(m, P), bass.ts(n, NQ)], ysb[:])

        def proj_extras(g):
            # groups 1/2 run inside attention(0)/(1) where Act has slack;
            # evict via Act there to relieve the saturated DVE
            oa = g <= 2
            ex = []
            for i in range(DG // P):
                ex.append(lambda i=i: emit_qk_tile(qT_sb, "q", i, g, on_act=oa))
            for i in range(DG // P):
                ex.append(lambda i=i: emit_qk_tile(kT_sb, "k", i, g, on_act=oa))
            for m in range(4 * g, 4 * g + 4):
                ex.append(lambda m=m: emit_v_tile(m, on_act=oa))
            return ex

        def outproj_extras(g):
            return [lambda m=m, n=n: emit_outproj_tile(m, n)
                    for m in range(4 * g, 4 * g + 4) for n in range(D // NQ)]

        def emit_pv(pos, hi, nch, c, qo, pT):
            for s in range(2):
                nc.tensor.matmul(
                    pos[:, s, qo:NQ], v_sb[:, c, 2 * hi + s, :],
                    pT[:, s, qo:NQ],
                    start=(c == 0), stop=(c == nch - 1))

        def emit_attention_group(j, extras, last=False, v_inline=None):
            """All 4 head-pairs for q-tile j, extras paced over the chunks."""
            nch = 4 * j + 4
            total_c = nch * (HPC // 2)
            n_extras = len(extras)
            ci = 0
            done_extras = 0
            for hi in range(HPC // 2):
                pos = pospool.tile([P, 2, NQ], mybir.dt.float32, tag="pos")
                pend = []  # (c, qo, pT)
                for c in range(nch):
                    qo = max(0, P * c - NQ * j)
                    ps = spool.tile([P, 2, NQ], mybir.dt.float32, tag="ps")
                    diag = c >= 4 * j
                    for s in range(2):
                        hb = s * HD
                        nc.tensor.matmul(
                            ps[:, s, qo:NQ],
                            kT_sb[hb:hb + HD, hi, bass.ts(c, P)],
                            qT_sb[hb:hb + HD, hi, NQ * j + qo:NQ * (j + 1)],
                            start=True, stop=True)
                    pT = pt.tile([P, 2, NQ], DT, tag="pT")
                    nc.scalar.activation(
                        pT[:, :, qo:NQ], ps[:, :, qo:NQ],
                        mybir.ActivationFunctionType.Exp,
                        scale=float(HD) ** -0.5 / SCL)
                    if diag:
                        # causal triangle on the diagonal block: in-place
                        # bf16 multiply on DVE (PV lags 2 chunks, so the
                        # cross-engine hop stays off the critical path)
                        for s in range(2):
                            nc.vector.tensor_tensor(
                                pT[:, s, qo:qo + P], pT[:, s, qo:qo + P],
                                mneg_sb[:], mybir.AluOpType.mult)
                    pend.append((c, qo, pT))
                    if len(pend) > 4:
                        emit_pv(pos, hi, nch, *pend.pop(0))
                    if v_inline and hi == 0 and c < len(v_inline):
                        v_inline[c]()
                    ci += 1
                    # pace extras to deplete slightly after the last chunk so
                    # a few remain to cover the final exp->PV drain
                    while extras and done_extras * (total_c + 20) < ci * n_extras:
                        extras.popleft()()
                        done_extras += 1
                while pend:
                    emit_pv(pos, hi, nch, *pend.pop(0))
                    if extras and hi == HPC // 2 - 1:
                        extras.popleft()()
                rcp = rc.tile([P, 2, NQ], mybir.dt.float32, tag="rcp")
                if last and hi == HPC // 2 - 1:
                    # tail: normalize straight from PSUM (shortest oT chain)
                    nc.vector.reciprocal(rcp[HD:P, :, :], pos[HD:P, :, :])
                    for s in range(2):
                        nc.vector.tensor_tensor(
                            oT_sb[s * HD:(s + 1) * HD, hi, bass.ts(j, NQ)],
                            pos[0:HD, s, :], rcp[HD:P, s, :], mybir.AluOpType.mult)
                else:
                    # free the pos bank fast: evict to SBUF, normalize there.
                    # rcp lands on partitions 0:HD so the SBUF+SBUF multiply
                    # reads both inputs at the same base partition (walrus
                    # requires equal SBUF base partitions).
                    posE = rc.tile([P, 2, NQ], mybir.dt.float32, tag="posE")
                    nc.vector.tensor_copy(posE[:], pos[:])
                    nc.vector.reciprocal(rcp[0:HD, :, :], posE[HD:P, :, :])
                    for s in range(2):
                        nc.vector.tensor_tensor(
                            oT_sb[s * HD:(s + 1) * HD, hi, bass.ts(j, NQ)],
                            posE[0:HD, s, :], rcp[0:HD, s, :], mybir.AluOpType.mult)
                if last and hi >= 2:
                    continue   # tail outproj reads oT (bf16) for hi 2/3
                eng = nc.gpsimd
                eng.tensor_copy(
                    oh8_sb[:, hi // 2, hi % 2, bass.ts(j, NQ)],
                    oT_sb[:, hi, bass.ts(j, NQ)])
                eng.tensor_tensor(
                    ol8_sb[:, hi // 2, hi % 2, bass.ts(j, NQ)],
                    oT_sb[:, hi, bass.ts(j, NQ)],
                    oh8_sb[:, hi // 2, hi % 2, bass.ts(j, NQ)],
                    mybir.AluOpType.subtract)
            while extras:
                extras.popleft()()

        # ---- interleaved schedule ----
        # proj(g+1) extras must finish inside attention(g); outproj extras are
        # free to defer, so they all go to attention(3) whose Act deficit is
        # largest. outproj(3) trails as the unavoidable tail.
        for i in range(DG // P):
            emit_qk_tile(qT_sb, "q", i, 0)
        for i in range(DG // P):
            emit_qk_tile(kT_sb, "k", i, 0)
        v0 = [lambda m=m: emit_v_tile(m) for m in range(4)]
        for g in range(NJ):
            extras = deque()
            if g + 1 < NJ:
                extras.extend(proj_extras(g + 1))
            if g == NJ - 1:
                for gg in range(NJ - 1):
                    extras.extend(outproj_extras(gg))
            emit_attention_group(g, extras, last=(g == NJ - 1),
                                 v_inline=v0 if g == 0 else None)
        for mi, m in enumerate(range(4 * (NJ - 1), 4 * NJ)):
            ysb2 = ys.tile([P, 2, NQ], mybir.dt.float16, tag="ysb2", name="ysb2")
            for n in range(D // NQ):
                ps = (spool if n else pqpool).tile(
                    [P, NQ], mybir.dt.float32, tag="ps" if n else "pp",
                    name="pst")
                k = 0
                for ot, wt in ((oh8_sb, wo8h_sb), (oh8_sb, wo8l_sb),
                               (ol8_sb, wo8h_sb)):
                    nc.tensor.matmul(
                        ps[:], ot[:, 0, :, bass.ts(m, P)],
                        wt[:, 0, :, bass.ts(n, NQ)],
                        start=(k == 0), stop=False,
                        perf_mode=mybir.MatmulPerfMode.DoubleRow)
                    k += 1
                for c in range(2, 4):
                    nc.tensor.matmul(
                        ps[:], oT_sb[:, c, bass.ts(m, P)],
                        wo23_sb[:, c - 2, bass.ts(n, NQ)],
                        start=False, stop=(c == 3))
                if n:
                    nc.scalar.mul(ysb2[:, n, :], ps[:], 1.0 / WS)
                else:
                    nc.vector.tensor_scalar_mul(ysb2[:, n, :], ps[:], 1.0 / WS)
                if mi == 3:   # last tile: per-half DMA so the first half
                    nc.sync.dma_start(   # transfers under the second evict
                        y[bass.ts(m, P), n * NQ:(n + 1) * NQ], ysb2[:, n, :])
            if mi < 3:
                eng = nc.scalar if mi % 2 else nc.sync
                eng.dma_start(y[bass.ts(m, P), :], ysb2[:])

    split_waits(nc)
    return nc


def kernel(x, Wq, Wk, Wv, Wo, bo):
    x, Wq, Wk, Wv, Wo, bo = (np.asarray(a, np.float32) for a in (x, Wq, Wk, Wv, Wo, bo))
    if "nc" not in _CACHE:
        _CACHE["nc"] = build()
    nc = _CACHE["nc"]

    # causal keep-mask for the diagonal block: tri[k, q] = 1 where q >= k
    mnegT = np.triu(np.ones((P, P), np.float32)).astype(NPDT)
    ident = np.eye(P, dtype=np.float32).astype(NPDT)

    def dr_pack(a):  # [1024, N] -> [128, 4, 2, N] with channel = c*256 + i*128 + p
        return np.ascontiguousarray(a.reshape(CP, 2, P, -1).transpose(2, 0, 1, 3))

    in_maps = []
    for core in range(8):
        b, gsl = core // 2, core % 2
        sl = slice(gsl * DG, (gsl + 1) * DG)
        xt = np.ascontiguousarray(x[b].T)
        xh8 = xt.astype(NPF8)
        xl8 = (xt - xh8.astype(np.float32)).astype(NPF8)
        wos = Wo[sl, :] * WS
        wo23_np = np.ascontiguousarray(
            wos[2 * P:4 * P].reshape(2, P, D).transpose(1, 0, 2)).astype(NPDT)
        woh = wos.astype(NPF8)
        wol = (wos - woh.astype(np.float32)).astype(NPF8)
        pk = lambda a: np.ascontiguousarray(a.reshape(2, 2, P, D).transpose(2, 0, 1, 3))
        m = {"wo8h": pk(woh), "wo8l": pk(wol), "wo23": wo23_np,
             "mnegT": mnegT, "ident": ident,
             "xh": dr_pack(xh8), "xl": dr_pack(xl8)}
        for nm, W in (("q", Wq), ("k", Wk), ("v", Wv)):
            ws = W[:, sl] * WS
            wh = ws.astype(NPF8)
            wl = (ws - wh.astype(np.float32)).astype(NPF8)
            m["w%sh" % nm] = dr_pack(wh)
            m["w%sl" % nm] = dr_pack(wl)
        in_maps.append(m)
    res = run_bass_kernel_spmd(nc, in_maps, list(range(8)))
    out = np.empty((B, S, D), np.float32)
    for b in range(B):
        out[b] = (res.results[2 * b]["y"].astype(np.float32)
                  + res.results[2 * b + 1]["y"].astype(np.float32) + bo)
    return out


# revision 69
# speedup vs baseline: 1.0018x; 1.0007x over previous
"""Causal self-attention Trainium2 kernel (B=4, S=2048, D=1024, H=16).

Sharding: 8 cores = 4 batches x 2 head-groups (8 heads each).
Megatron-style: column-parallel QKV, row-parallel output projection;
the 2-way partial-sum reduce + bias happens on host at gather time.

Schedule: one interleaved PE stream. Attention for q-tile j is
software-pipelined (PE issues scores(c)/mask(c) ahead, PV lags 2
chunks so it never waits on the Act-engine exp), and the projection
tiles for group j+1 plus the output-projection tiles for group j-1
are sprinkled between attention chunks to absorb the PE slack while
Act (the per-chunk bottleneck) streams exps. Causal masking is a
-1e9 matmul accumulated into the score PSUM before the exp; softmax
denominators ride along PV as an appended ones-column block.

QKV projections run as fp8(e4m3) DoubleRow matmuls with error
feedback: x and 32*W are each split hi+lo fp8 on host and the three
significant cross terms accumulate in PSUM (xh*Wh + xh*Wl + xl*Wh),
contracting 256 channels per instruction at 0.5 cycles/row.
"""
import numpy as np
import ml_dtypes
from collections import deque
from contextlib import ExitStack

import concourse.bass as bass
import concourse.tile as tile
import concourse.mybir as mybir
from concourse.bass_utils import run_bass_kernel_spmd

B, S, D, H = 4, 2048, 1024, 16
HD = 64          # head dim
HPC = 8          # heads per core
DG = HPC * HD    # 512 dims per head-group
P = 128
NQ = 512         # q-tile width
NCH = S // P     # 16 k-chunks
NJ = S // NQ     # 4 q-tiles (= j-groups)
CP = 4           # 256-channel chunk-pairs over D (fp8 DoubleRow)
DT = mybir.dt.bfloat16
F8 = mybir.dt.float8e4
NPDT = ml_dtypes.bfloat16
NPF8 = ml_dtypes.float8_e4m3

WS = 32.0               # fp8 weight pre-scale (host side)
SCL = WS * WS           # scores carry WS^2; exp scale divides it out

_CACHE = {}


def split_waits(nc, maxw=1):
    """walrus here accepts at most 1 sync-wait per instruction; split extras onto NOPs."""
    for fn in nc.m.functions:
        for bb in fn.blocks:
            insts = list(bb.instructions)
            new_list = []
            changed = False
            for inst in insts:
                si = inst.sync_info
                waits = list(si.on_wait) if si and si.on_wait else []
                if len(waits) > maxw:
                    changed = True
                    head, keep = waits[:-maxw], waits[-maxw:]
                    for i in range(0, len(head), maxw):
                        nop = mybir.InstNoOp(
                            name=f"{inst.name}_wsplit{i}",
                            sync_info=mybir.SyncInfo(on_wait=head[i:i + maxw], on_update=[]),
                            bass_nofuse=True, engine=inst.engine)
                        nc.register_instruction(nop)
                        new_list.append(nop)
                    inst.sync_info = mybir.SyncInfo(
                        on_wait=keep,
                        on_update=list(si.on_update) if si.on_update else [])
                new_list.append(inst)
            if changed:
                bb.instructions = new_list


def build():
    nc = bass.Bass(trn_type="TRN2", target_bir_lowering=False, debug=False)
    xh = nc.dram_tensor("xh", [P, CP, 2, S], F8, kind="ExternalInput").ap()
    xl = nc.dram_tensor("xl", [P, CP, 2, S], F8, kind="ExternalInput").ap()
    wqkv = {}
    for nm in ("wqh", "wql", "wkh", "wkl", "wvh", "wvl"):
        wqkv[nm] = nc.dram_tensor(nm, [P, CP, 2, DG], F8, kind="ExternalInput").ap()
    wo8h = nc.dram_tensor("wo8h", [P, 2, 2, D], F8, kind="ExternalInput").ap()
    wo8l = nc.dram_tensor("wo8l", [P, 2, 2, D], F8, kind="ExternalInput").ap()
    wo23 = nc.dram_tensor("wo23", [P, 2, D], DT, kind="ExternalInput").ap()
    mnegT = nc.dram_tensor("mnegT", [P, P], DT, kind="ExternalInput").ap()
    ident = nc.dram_tensor("ident", [P, P], DT, kind="ExternalInput").ap()
    y = nc.dram_tensor("y", [S, D], mybir.dt.float16, kind="ExternalOutput").ap()

    with tile.TileContext(nc) as tc, ExitStack() as ctx:
        const = ctx.enter_context(tc.tile_pool(name="const", bufs=1))
        xw = ctx.enter_context(tc.tile_pool(name="xw", bufs=1))
        acts = ctx.enter_context(tc.tile_pool(name="acts", bufs=1))

        # ---- resident inputs; DMA order gates the pipeline fill ----
        xh_sb = xw.tile([P, CP, 2, S], F8)
        xl_sb = xw.tile([P, CP, 2, S], F8)
        w_sb = {}

        def load_w(nm):
            w_sb[nm] = xw.tile([P, CP, 2, DG], F8, name=nm)
            nc.sync.dma_start(w_sb[nm][:], wqkv[nm][:])

        def load_xq(g):  # x S-quarter g: all group-g projection inputs
            for c in range(CP):
                nc.sync.dma_start(xh_sb[:, c, :, bass.ts(g, NQ)], xh[:, c, :, bass.ts(g, NQ)])
                nc.sync.dma_start(xl_sb[:, c, :, bass.ts(g, NQ)], xl[:, c, :, bass.ts(g, NQ)])

        load_w("wqh")
        for c in range(CP):
            nc.sync.dma_start(xh_sb[:, c, :, 0:NQ], xh[:, c, :, 0:NQ])
        load_w("wql")
        for c in range(CP):
            nc.sync.dma_start(xl_sb[:, c, :, 0:NQ], xl[:, c, :, 0:NQ])
        for nm in ("wkh", "wkl", "wvh", "wvl"):
            load_w(nm)
        mneg_sb = const.tile([P, P], DT)
        nc.sync.dma_start(mneg_sb[:], mnegT[:])
        id_sb = const.tile([P, P], DT)
        nc.sync.dma_start(id_sb[:], ident[:])
        for g in range(1, NJ):    # remaining x quarters stream under compute
            load_xq(g)
        wo8h_sb = xw.tile([P, 2, 2, D], F8)
        nc.sync.dma_start(wo8h_sb[:], wo8h[:])
        wo8l_sb = xw.tile([P, 2, 2, D], F8)
        nc.sync.dma_start(wo8l_sb[:], wo8l[:])
        wo23_sb = xw.tile([P, 2, D], DT)
        nc.sync.dma_start(wo23_sb[:], wo23[:])

        # ---- resident activations ----
        qT_sb = acts.tile([P, DG // P, S], DT)   # [2-head block, hi, s]
        kT_sb = acts.tile([P, DG // P, S], DT)
        v_sb = acts.tile([P, NCH, HPC, P], DT)   # [k part, chunk, head, V|ones]
        nc.vector.memset(v_sb[:, :, :, HD:], 1.0)
        oT_sb = acts.tile([P, DG // P, S], DT)
        oh8_sb = acts.tile([P, 2, 2, S], F8)   # DoubleRow lhsT: dg = c2*256+i*128+p
        ol8_sb = acts.tile([P, 2, 2, S], F8)

        spool = ctx.enter_context(tc.tile_pool(name="sp", bufs=2, space="PSUM"))
        pqpool = ctx.enter_context(tc.tile_pool(name="pq", bufs=2, space="PSUM"))
        pospool = ctx.enter_context(tc.tile_pool(name="po", bufs=1, space="PSUM"))
        pt = ctx.enter_context(tc.tile_pool(name="pt", bufs=6))
        rc = ctx.enter_context(tc.tile_pool(name="rc", bufs=1))
        ys = ctx.enter_context(tc.tile_pool(name="ys", bufs=6))

        def emit_qk_tile(dst_sb, kind, i, g, on_act=False):
            """One [128, 512] tile of Q^T/K^T (d-block i, q-cols of group g)."""
            ps = pqpool.tile([P, NQ], mybir.dt.float32, tag="pp")
            wh, wl = w_sb["w%sh" % kind], w_sb["w%sl" % kind]
            k = 0
            for wt, xt in ((wh, xh_sb), (wl, xh_sb), (wh, xl_sb)):
                for c in range(CP):
                    nc.tensor.matmul(
                        ps[:], wt[:, c, :, bass.ts(i, P)],
                        xt[:, c, :, bass.ts(g, NQ)],
                        start=(k == 0), stop=(k == 3 * CP - 1),
                        perf_mode=mybir.MatmulPerfMode.DoubleRow)
                    k += 1
            if on_act:
                nc.scalar.copy(dst_sb[:, i, bass.ts(g, NQ)], ps[:])
            else:
                nc.vector.tensor_copy(dst_sb[:, i, bass.ts(g, NQ)], ps[:])

        def emit_v_tile(m, on_act=False):
            """V s-block m -> v_sb[:, m, :, 0:HD] (scaled back by 1/WS)."""
            ps = pqpool.tile([P, DG], mybir.dt.float32, tag="pp")
            wh, wl = w_sb["wvh"], w_sb["wvl"]
            k = 0
            for xt, wt in ((xh_sb, wh), (xh_sb, wl), (xl_sb, wh)):
                for c in range(CP):
                    nc.tensor.matmul(
                        ps[:], xt[:, c, :, bass.ts(m, P)], wt[:, c],
                        start=(k == 0), stop=(k == 3 * CP - 1),
                        perf_mode=mybir.MatmulPerfMode.DoubleRow)
                    k += 1
            if on_act:
                nc.scalar.mul(v_sb[:, m, :, 0:HD],
                              ps[:].rearrange("p (h d) -> p h d", d=HD), 1.0 / WS)
            else:
                nc.vector.tensor_scalar_mul(
                    v_sb[:, m, :, 0:HD],
                    ps[:].rearrange("p (h d) -> p h d", d=HD), 1.0 / WS)

        def emit_outproj_tile(m, n, on_act=False, pool=None):
            ps = (pool or pqpool).tile([P, NQ], mybir.dt.float32,
                                       tag="pp" if pool is None else "ps")
            k = 0
            for c2 in range(2):
                for ot, wt in ((oh8_sb, wo8h_sb), (oh8_sb, wo8l_sb),
                               (ol8_sb, wo8h_sb)):
                    nc.tensor.matmul(
                        ps[:], ot[:, c2, :, bass.ts(m, P)],
                        wt[:, c2, :, bass.ts(n, NQ)],
                        start=(k == 0), stop=(k == 5),
                        perf_mode=mybir.MatmulPerfMode.DoubleRow)
                    k += 1
            ysb = ys.tile([P, NQ], mybir.dt.float16, tag="ysb")
            if on_act:   # tail: Act is idle once the exps are done
                nc.scalar.mul(ysb[:], ps[:], 1.0 / WS)
            else:
                nc.vector.tensor_scalar_mul(ysb[:], ps[:], 1.0 / WS)
            nc.sync.dma_start(y[bass.ts(m, P), bass.ts(n, NQ)], ysb[:])

        def proj_extras(g):
            # groups 1/2 run inside attention(0)/(1) where Act has slack;
            # evict via Act there to relieve the saturated DVE
            oa = g <= 2
            ex = []
            for i in range(DG // P):
                ex.append(lambda i=i: emit_qk_tile(qT_sb, "q", i, g, on_act=oa))
            for i in range(DG // P):
                ex.append(lambda i=i: emit_qk_tile(kT_sb, "k", i, g, on_act=oa))
            for m in range(4 * g, 4 * g + 4):
                ex.append(lambda m=m: emit_v_tile(m, on_act=oa))
            return ex

        def outproj_extras(g):
            return [lambda m=m, n=n: emit_outproj_tile(m, n)
                    for m in range(4 * g, 4 * g + 4) for n in range(D // NQ)]

        def emit_pv(pos, hi, nch, c, qo, pT):
            for s in range(2):
                nc.tensor.matmul(
                    pos[:, s, qo:NQ], v_sb[:, c, 2 * hi + s, :],
                    pT[:, s, qo:NQ],
                    start=(c == 0), stop=(c == nch - 1))

        def emit_attention_group(j, extras, last=False, v_inline=None):
            """All 4 head-pairs for q-tile j, extras paced over the chunks."""
            nch = 4 * j + 4
            total_c = nch * (HPC // 2)
            n_extras = len(extras)
            ci = 0
            done_extras = 0
            for hi in range(HPC // 2):
                pos = pospool.tile([P, 2, NQ], mybir.dt.float32, tag="pos")
                pend = []  # (c, qo, pT)
                for c in range(nch):
                    qo = max(0, P * c - NQ * j)
                    ps = spool.tile([P, 2, NQ], mybir.dt.float32, tag="ps")
                    diag = c >= 4 * j
                    for s in range(2):
                        hb = s * HD
                        nc.tensor.matmul(
                            ps[:, s, qo:NQ],
                            kT_sb[hb:hb + HD, hi, bass.ts(c, P)],
                            qT_sb[hb:hb + HD, hi, NQ * j + qo:NQ * (j + 1)],
                            start=True, stop=True)
                    pT = pt.tile([P, 2, NQ], DT, tag="pT")
                    nc.scalar.activation(
                        pT[:, :, qo:NQ], ps[:, :, qo:NQ],
                        mybir.ActivationFunctionType.Exp,
                        scale=float(HD) ** -0.5 / SCL)
                    if diag:
                        # causal triangle on the diagonal block: in-place
                        # bf16 multiply on DVE (PV lags 2 chunks, so the
                        # cross-engine hop stays off the critical path)
                        for s in range(2):
                            nc.vector.tensor_tensor(
                                pT[:, s, qo:qo + P], pT[:, s, qo:qo + P],
                                mneg_sb[:], mybir.AluOpType.mult)
                    pend.append((c, qo, pT))
                    if len(pend) > 4:
                        emit_pv(pos, hi, nch, *pend.pop(0))
                    if v_inline and hi == 0 and c < len(v_inline):
                        v_inline[c]()
                    ci += 1
                    # pace extras to deplete slightly after the last chunk so
                    # a few remain to cover the final exp->PV drain
                    while extras and done_extras * (total_c + 20) < ci * n_extras:
                        extras.popleft()()
                        done_extras += 1
                while pend:
                    emit_pv(pos, hi, nch, *pend.pop(0))
                    if extras and hi == HPC // 2 - 1:
                        extras.popleft()()
                rcp = rc.tile([P, 2, NQ], mybir.dt.float32, tag="rcp")
                if last and hi == HPC // 2 - 1:
                    # tail: normalize straight from PSUM (shortest oT chain)
                    nc.vector.reciprocal(rcp[HD:P, :, :], pos[HD:P, :, :])
                    for s in range(2):
                        nc.vector.tensor_tensor(
                            oT_sb[s * HD:(s + 1) * HD, hi, bass.ts(j, NQ)],
                            pos[0:HD, s, :], rcp[HD:P, s, :], mybir.AluOpType.mult)
                else:
                    # free the pos bank fast: evict to SBUF, normalize there.
                    # rcp lands on partitions 0:HD so the SBUF+SBUF multiply
                    # reads both inputs at the same base partition (walrus
                    # requires equal SBUF base partitions).
                    posE = rc.tile([P, 2, NQ], mybir.dt.float32, tag="posE")
                    nc.vector.tensor_copy(posE[:], pos[:])
                    nc.vector.reciprocal(rcp[0:HD, :, :], posE[HD:P, :, :])
                    for s in range(2):
                        nc.vector.tensor_tensor(
                            oT_sb[s * HD:(s + 1) * HD, hi, bass.ts(j, NQ)],
                            posE[0:HD, s, :], rcp[0:HD, s, :], mybir.AluOpType.mult)
                if last and hi >= 2:
                    continue   # tail outproj reads oT (bf16) for hi 2/3
                eng = nc.gpsimd
                eng.tensor_copy(
                    oh8_sb[:, hi // 2, hi % 2, bass.ts(j, NQ)],
                    oT_sb[:, hi, bass.ts(j, NQ)])
                eng.tensor_tensor(
                    ol8_sb[:, hi // 2, hi % 2, bass.ts(j, NQ)],
                    oT_sb[:, hi, bass.ts(j, NQ)],
                    oh8_sb[:, hi // 2, hi % 2, bass.ts(j, NQ)],
                    mybir.AluOpType.subtract)
            while extras:
                extras.popleft()()

        # ---- interleaved schedule ----
        # proj(g+1) extras must finish inside attention(g); outproj extras are
        # free to defer, so they all go to attention(3) whose Act deficit is
        # largest. outproj(3) trails as the unavoidable tail.
        for i in range(DG // P):
            emit_qk_tile(qT_sb, "q", i, 0)
        for i in range(DG // P):
            emit_qk_tile(kT_sb, "k", i, 0)
        v0 = [lambda m=m: emit_v_tile(m) for m in range(4)]
        for g in range(NJ):
            extras = deque()
            if g + 1 < NJ:
                extras.extend(proj_extras(g + 1))
            if g == NJ - 1:
                for gg in range(NJ - 1):
                    extras.extend(outproj_extras(gg))
            emit_attention_group(g, extras, last=(g == NJ - 1),
                                 v_inline=v0 if g == 0 else None)
        for mi, m in enumerate(range(4 * (NJ - 1), 4 * NJ)):
            ysb2 = ys.tile([P, 2, NQ], mybir.dt.float16, tag="ysb2", name="ysb2")
            for n in range(D // NQ):
                ps = (spool if n else pqpool).tile(
                    [P, NQ], mybir.dt.float32, tag="ps" if n else "pp",
                    name="pst")
                k = 0
                for ot, wt in ((oh8_sb, wo8h_sb), (oh8_sb, wo8l_sb),
                               (ol8_sb, wo8h_sb)):
                    nc.tensor.matmul(
                        ps[:], ot[:, 0, :, bass.ts(m, P)],
                        wt[:, 0, :, bass.ts(n, NQ)],
                        start=(k == 0), stop=False,
                        perf_mode=mybir.MatmulPerfMode.DoubleRow)
                    k += 1
                for c in range(2, 4):
                    nc.tensor.matmul(
                        ps[:], oT_sb[:, c, bass.ts(m, P)],
                        wo23_sb[:, c - 2, bass.ts(n, NQ)],
                        start=False, stop=(c == 3))
                if n:
                    nc.scalar.mul(ysb2[:, n, :], ps[:], 1.0 / WS)
                else:
                    nc.vector.tensor_scalar_mul(ysb2[:, n, :], ps[:], 1.0 / WS)
                if mi == 3:   # last tile: per-half DMA so the first half
                    nc.sync.dma_start(   # transfers under the second evict
                        y[bass.ts(m, P), n * NQ:(n + 1) * NQ], ysb2[:, n, :])
            if mi < 3:
                eng = nc.scalar if mi % 2 else nc.sync
                eng.dma_start(y[bass.ts(m, P), :], ysb2[:])

    split_waits(nc)
    return nc


def kernel(x, Wq, Wk, Wv, Wo, bo):
    x, Wq, Wk, Wv, Wo, bo = (np.asarray(a, np.float32) for a in (x, Wq, Wk, Wv, Wo, bo))
    if "nc" not in _CACHE:
        _CACHE["nc"] = build()
    nc = _CACHE["nc"]

    # causal keep-mask for the diagonal block: tri[k, q] = 1 where q >= k
    mnegT = np.triu(np.ones((P, P), np.float32)).astype(NPDT)
    ident = np.eye(P, dtype=np.float32).astype(NPDT)

    def dr_pack(a):  # [1024, N] -> [128, 4, 2, N] with channel = c*256 + i*128 + p
        return np.ascontiguousarray(a.reshape(CP, 2, P, -1).transpose(2, 0, 1, 3))

    in_maps = []
    for core in range(8):
        b, gsl = core // 2, core % 2
        sl = slice(gsl * DG, (gsl + 1) * DG)
        xt = np.ascontiguousarray(x[b].T)
        xh8 = xt.astype(NPF8)
        xl8 = (xt - xh8.astype(np.float32)).astype(NPF8)
        wos = Wo[sl, :] * WS
        wo23_np = np.ascontiguousarray(
            wos[2 * P:4 * P].reshape(2, P, D).transpose(1, 0, 2)).astype(NPDT)
        woh = wos.astype(NPF8)
        wol = (wos - woh.astype(np.float32)).astype(NPF8)
        pk = lambda a: np.ascontiguousarray(a.reshape(2, 2, P, D).transpose(2, 0, 1, 3))
        m = {"wo8h": pk(woh), "wo8l": pk(wol), "wo23": wo23_np,
             "mnegT": mnegT, "ident": ident,
             "xh": dr_pack(xh8), "xl": dr_pack(xl8)}
        for nm, W in (("q", Wq), ("k", Wk), ("v", Wv)):
            ws = W[:, sl] * WS
            wh = ws.astype(NPF8)
            wl = (ws - wh.astype(np.float32)).astype(NPF8)
            m["w%sh" % nm] = dr_pack(wh)
            m["w%sl" % nm] = dr_pack(wl)
        in_maps.append(m)
    res = run_bass_kernel_spmd(nc, in_maps, list(range(8)))
    out = np.empty((B, S, D), np.float32)
    for b in range(B):
        out[b] = (res.results[2 * b]["y"].astype(np.float32)
                  + res.results[2 * b + 1]["y"].astype(np.float32) + bo)
    return out
